# revision 10
# baseline (speedup 1.0000x reference)
"""v9: fp32r upper-triangle Gram; fp8e4 256*(G-thr); consolidated out DMA.

Per batch, G = xn^T xn is symmetric: only upper-triangular [128,512] tiles
are computed (host mirrors). The two cores of a batch split every column
chunk's four 128-row blocks {0,1}/{2,3}; a per-core within-chunk block
permutation ([0,1,2,3] / [2,3,0,1]) makes the SPMD program identical.

Matmuls are single-pass float32r (~12-13 mantissa bits). The device
outputs fp8e4 of 256*(G - thr): sign = adjacency, magnitude flags the
|G-thr| < 1e-4 band the host recomputes exactly in float64.

The per-tile PSUM drain is split across the Vector and Scalar engines so
it stays under the 3-matmul group time (pipeline is PE-paced). Each
column-chunk phase accumulates its tiles in one SBUF buffer that is
written out with two large partition-contiguous DMAs (the sync DMA queue
costs ~600ns per instruction regardless of size, so 16 big output DMAs
instead of 72 small ones keeps the queue far from saturation). Input
chunk DMAs are emitted interleaved with compute; a garbage-matmul warmup
keeps the PE clock-gate warm before real work.
"""

import sys

for _p in ("/opt/trn_rl_repo", "/root/.axon_site/_ro/trn_rl_repo"):
    if _p not in sys.path:
        sys.path.append(_p)

import numpy as np

B, C, N = 4, 384, 4096
HALF = N // 2
KT = C // 128          # 3 contraction tiles
NCHUNK = 8             # 512-wide column chunks
CW = 512
NCORES = 2 * B
PPF_09 = 1.2815515655446004
EPS = 1e-12
MBITS = 13             # fp32r mantissa grid (pre-round on host)
MARGIN = 1e-4          # |G - thr| band recomputed exactly on host
OSCALE = 256.0         # output = fp8e4( OSCALE * (G - thr) )
NWARM = 14             # PE warmup matmuls (run during input DMA wait)

_compiled_nc = None


def _build_nc():
    import concourse.bacc as bacc
    import concourse.tile as tile
    import concourse.mybir as mybir

    f32 = mybir.dt.float32
    f32r = mybir.dt.float32r
    f8 = mybir.dt.float8e4
    bf16 = mybir.dt.bfloat16
    Alu = mybir.AluOpType
    Act = mybir.ActivationFunctionType

    nc = bacc.Bacc("TRN2", target_bir_lowering=False, debug=False)

    x_d = nc.dram_tensor("xc", [NCHUNK, 128, KT, CW], f32r,
                         kind="ExternalInput")
    nthr_d = nc.dram_tensor("nthr", [128, 1], f32, kind="ExternalInput")
    # [m, row-in-block, rb-slot, col]: partition-major so each output DMA
    # moves multi-KB contiguous runs per partition
    d_d = nc.dram_tensor("d", [NCHUNK, 128, 16, CW], f8,
                         kind="ExternalOutput")

    with tile.TileContext(nc) as tc:
        with tc.tile_pool(name="xpool", bufs=1) as xpool, \
             tc.tile_pool(name="psum", bufs=5, space="PSUM") as psum, \
             tc.tile_pool(name="warmp", bufs=1, space="PSUM") as warmp, \
             tc.tile_pool(name="mpool", bufs=2) as mpool:
            nthr_t = xpool.tile([128, 1], f32, name="nthr_t")
            nc.sync.dma_start(out=nthr_t[:], in_=nthr_d.ap())
            dummy = xpool.tile([128, CW], bf16, name="dummy")
            nc.vector.memset(dummy[:], 1.0)
            xc = [xpool.tile([128, KT, CW], f32r, name=f"xc{c}")
                  for c in range(NCHUNK)]
            # chunk 0 split per k so the first matmul starts earliest
            for k in range(KT):
                nc.sync.dma_start(out=xc[0][:, k, :], in_=x_d.ap()[0, :, k, :])
            nc.sync.dma_start(out=xc[1][:], in_=x_d[1])

            # HAM warmup: garbage matmuls into a scratch bank while the
            # input stream lands; results are never read.
            wps = warmp.tile([128, CW], f32, name="wps")
            for i in range(NWARM):
                nc.tensor.matmul(wps[:], dummy[:, 0:128], dummy[:],
                                 start=True, stop=True)

            # wavefront over moving chunks: tiles for chunk m need only
            # chunks q <= m; chunk m+2's DMA is emitted between phases so
            # input DMAs interleave with output DMAs on the queue.
            for m in range(NCHUNK):
                if m + 2 < NCHUNK:
                    nc.sync.dma_start(out=xc[m + 2][:], in_=x_d[m + 2])
                T = 2 * (m + 1)
                dt = mpool.tile([128, T, CW], f8, name="dt")
                for rb in range(T):
                    q, r = rb // 2, rb % 2
                    ps = psum.tile([128, CW], f32, name="ps")
                    for k in range(KT):
                        nc.tensor.matmul(
                            ps[:],
                            xc[q][:, k, r * 128:(r + 1) * 128],
                            xc[m][:, k, :],
                            start=(k == 0), stop=(k == KT - 1),
                        )
                    nc.vector.tensor_scalar(
                        dt[:, rb, 0:256], ps[:, 0:256], OSCALE, nthr_t[:],
                        op0=Alu.mult, op1=Alu.add)
                    nc.scalar.activation(
                        dt[:, rb, 256:CW], ps[:, 256:CW], Act.Identity,
                        bias=nthr_t[:], scale=OSCALE)
                    if rb == T // 2 - 1:
                        nc.sync.dma_start(
                            out=d_d.ap()[m][:, 0:T // 2, :],
                            in_=dt[:, 0:T // 2, :])
                nc.sync.dma_start(
                    out=d_d.ap()[m][:, T // 2:T, :],
                    in_=dt[:, T // 2:T, :])
    nc.compile()
    return nc


def get_nc():
    global _compiled_nc
    if _compiled_nc is None:
        _compiled_nc = _build_nc()
    return _compiled_nc


def _round_mant(x, bits):
    """Round fp32 array to `bits` explicit mantissa bits."""
    m, e = np.frexp(x)
    s = np.float32(1 << bits)
    m = np.round(m * s) / s
    return np.ldexp(m, e).astype(np.float32)


_PERM = ([0, 1, 2, 3], [2, 3, 0, 1])   # self-inverse block perms per h

_state = {}


def make_inputs(x):
    xs = np.asarray(x)[:, :, :, 0]                      # (B, C, N) fp32
    nrm = np.sqrt(np.sum(xs * xs, axis=1, keepdims=True))
    xn = xs / np.maximum(nrm, EPS)

    Nsq = float(N) * float(N)
    in_maps = []
    xn64s, thrs = [], []
    for b in range(B):
        xb64 = xn[b].astype(np.float64)
        s = xb64.sum(axis=1)
        M = xb64 @ xb64.T
        sum_g = float(s @ s)
        sum_g2 = float((M * M).sum())
        mean = (2.0 * sum_g - 2.0 * Nsq) / Nsq
        s2 = 4.0 * sum_g2 - 8.0 * sum_g + 4.0 * Nsq
        var = (s2 - Nsq * mean * mean) / (Nsq - 1.0)
        t_b = (mean + PPF_09 * np.sqrt(var) + 2.0) / 2.0
        xn64s.append(xb64)
        thrs.append(t_b)

        nthr_dev = np.full((128, 1), -t_b * OSCALE, np.float32)
        xbr = _round_mant(xn[b].astype(np.float32), MBITS)  # (C, N)
        for h in range(2):
            xloc = xbr.reshape(C, NCHUNK, 4, 128)[:, :, _PERM[h], :]
            xloc = xloc.reshape(C, N)
            xcarr = xloc.reshape(KT, 128, NCHUNK, CW).transpose(2, 1, 0, 3)
            in_maps.append({
                "xc": np.ascontiguousarray(xcarr),
                "nthr": nthr_dev,
            })
    _state["xn64"] = xn64s
    _state["thr"] = thrs
    return in_maps


def assemble(results):
    out = np.empty((2, B * N * N), np.int32)
    iota = np.arange(N, dtype=np.int32)
    neg1 = np.int32(-1)
    for b in range(B):
        dU = np.empty((N, N), np.float32)
        for h in range(2):
            dv = results[2 * b + h]["d"]      # [m, 128, slot, 512] fp8
            for m in range(NCHUNK):
                T = 2 * (m + 1)
                blk = dv[m, :, 0:T, :].astype(np.float32)  # [128, T, 512]
                if h == 1:
                    blk = blk.reshape(128, T, 4, 128)[:, :, _PERM[1], :]
                    blk = blk.reshape(128, T, CW)
                for rb in range(T):
                    a = 4 * (rb // 2) + (rb % 2) + 2 * h
                    dU[a * 128:(a + 1) * 128,
                       m * CW:(m + 1) * CW] = blk[:, rb, :]
        adjU = (dU > 0).astype(np.uint8)
        nearU = np.triu(np.abs(dU) < MARGIN * OSCALE)
        ii, jj = np.nonzero(nearU)
        if ii.size:
            xn64 = _state["xn64"][b]
            g = np.einsum('ci,ci->i', xn64[:, ii], xn64[:, jj])
            adjU[ii, jj] = g > _state["thr"][b]
        adj = np.triu(adjU)
        adj += np.triu(adjU, 1).T
        src = b * N + iota
        out[0, b * N * N:(b + 1) * N * N] = np.where(
            adj, src[:, None], neg1).ravel()
        out[1, b * N * N:(b + 1) * N * N] = np.where(
            adj, src[None, :], neg1).ravel()
    return out


def kernel(x):
    from concourse.bass_utils import run_bass_kernel_spmd

    nc = get_nc()
    in_maps = make_inputs(x)
    res = run_bass_kernel_spmd(nc, in_maps, list(range(NCORES)))
    return assemble(res.results)


# revision 11
# speedup vs baseline: 1.1109x; 1.1109x over previous
"""v9: fp32r upper-triangle Gram; fp8e4 256*(G-thr); consolidated out DMA.

Per batch, G = xn^T xn is symmetric: only upper-triangular [128,512] tiles
are computed (host mirrors). The two cores of a batch split every column
chunk's four 128-row blocks {0,1}/{2,3}; a per-core within-chunk block
permutation ([0,1,2,3] / [2,3,0,1]) makes the SPMD program identical.

Matmuls are single-pass float32r (~12-13 mantissa bits). The device
outputs fp8e4 of 256*(G - thr): sign = adjacency, magnitude flags the
|G-thr| < 1e-4 band the host recomputes exactly in float64.

The per-tile PSUM drain is split across the Vector and Scalar engines so
it stays under the 3-matmul group time (pipeline is PE-paced). Each
column-chunk phase accumulates its tiles in one SBUF buffer that is
written out with two large partition-contiguous DMAs (the sync DMA queue
costs ~600ns per instruction regardless of size, so 16 big output DMAs
instead of 72 small ones keeps the queue far from saturation). Input
chunk DMAs are emitted interleaved with compute; a garbage-matmul warmup
keeps the PE clock-gate warm before real work.
"""

import sys

for _p in ("/opt/trn_rl_repo", "/root/.axon_site/_ro/trn_rl_repo"):
    if _p not in sys.path:
        sys.path.append(_p)

import numpy as np

B, C, N = 4, 384, 4096
HALF = N // 2
KT = C // 128          # 3 contraction tiles
NCHUNK = 8             # 512-wide column chunks
CW = 512
NCORES = 2 * B
PPF_09 = 1.2815515655446004
EPS = 1e-12
MBITS = 13             # fp32r mantissa grid (pre-round on host)
MARGIN = 1e-4          # |G - thr| band recomputed exactly on host
OSCALE = 256.0         # output = fp8e4( OSCALE * (G - thr) )
NWARM = 9              # PE warmup matmuls (run during input DMA wait)

_compiled_nc = None


def _build_nc():
    import concourse.bacc as bacc
    import concourse.tile as tile
    import concourse.mybir as mybir

    f32 = mybir.dt.float32
    f32r = mybir.dt.float32r
    f8 = mybir.dt.float8e4
    bf16 = mybir.dt.bfloat16
    Alu = mybir.AluOpType
    Act = mybir.ActivationFunctionType

    nc = bacc.Bacc("TRN2", target_bir_lowering=False, debug=False)

    x_d = nc.dram_tensor("xc", [NCHUNK, 128, KT, CW], f32r,
                         kind="ExternalInput")
    nthr_d = nc.dram_tensor("nthr", [128, 1], f32, kind="ExternalInput")
    # [m, row-in-block, rb-slot, col]: partition-major so each output DMA
    # moves multi-KB contiguous runs per partition
    d_d = nc.dram_tensor("d", [NCHUNK, 128, 16, CW], f8,
                         kind="ExternalOutput")

    with tile.TileContext(nc) as tc:
        with tc.tile_pool(name="xpool", bufs=1) as xpool, \
             tc.tile_pool(name="psum", bufs=6, space="PSUM") as psum, \
             tc.tile_pool(name="warmp", bufs=1, space="PSUM") as warmp, \
             tc.tile_pool(name="mpool", bufs=2) as mpool:
            nthr_t = xpool.tile([128, 1], f32, name="nthr_t")
            nc.sync.dma_start(out=nthr_t[:], in_=nthr_d.ap())
            dummy = xpool.tile([128, CW], bf16, name="dummy")
            nc.vector.memset(dummy[:], 1.0)
            xc = [xpool.tile([128, KT, CW], f32r, name=f"xc{c}")
                  for c in range(NCHUNK)]
            # chunk 0 split per k so the first matmul starts earliest
            for k in range(KT):
                nc.sync.dma_start(out=xc[0][:, k, :], in_=x_d.ap()[0, :, k, :])
            nc.sync.dma_start(out=xc[1][:], in_=x_d[1])

            # HAM warmup: garbage matmuls into a scratch bank while the
            # input stream lands; results are never read.
            wps = warmp.tile([128, CW], f32, name="wps")
            for i in range(NWARM):
                nc.tensor.matmul(wps[:], dummy[:, 0:128], dummy[:],
                                 start=True, stop=True)

            # wavefront over moving chunks: tiles for chunk m need only
            # chunks q <= m; chunk m+2's DMA is emitted between phases so
            # input DMAs interleave with output DMAs on the queue.
            for m in range(NCHUNK):
                if m + 2 < NCHUNK:
                    nc.sync.dma_start(out=xc[m + 2][:], in_=x_d[m + 2])
                T = 2 * (m + 1)
                # separate per-engine output buffers so the two drain
                # engines never serialize on a shared tile: even rb ->
                # Vector (slots 0..7), odd rb -> Scalar (slots 8..15)
                dvb = mpool.tile([128, m + 1, CW], f8, name="dvb")
                acb = mpool.tile([128, m + 1, CW], f8, name="acb")
                for rb in range(T):
                    q, r = rb // 2, rb % 2
                    ps = psum.tile([128, CW], f32, name="ps")
                    for k in range(KT):
                        nc.tensor.matmul(
                            ps[:],
                            xc[q][:, k, r * 128:(r + 1) * 128],
                            xc[m][:, k, :],
                            start=(k == 0), stop=(k == KT - 1),
                        )
                    if rb % 2 == 0:
                        nc.vector.tensor_scalar(
                            dvb[:, rb // 2, :], ps[:], OSCALE, nthr_t[:],
                            op0=Alu.mult, op1=Alu.add)
                    else:
                        nc.scalar.activation(
                            acb[:, rb // 2, :], ps[:], Act.Identity,
                            bias=nthr_t[:], scale=OSCALE)
                nc.sync.dma_start(
                    out=d_d.ap()[m][:, 0:m + 1, :], in_=dvb[:])
                nc.sync.dma_start(
                    out=d_d.ap()[m][:, 8:8 + m + 1, :], in_=acb[:])
    nc.compile()
    return nc


def get_nc():
    global _compiled_nc
    if _compiled_nc is None:
        _compiled_nc = _build_nc()
    return _compiled_nc


def _round_mant(x, bits):
    """Round fp32 array to `bits` explicit mantissa bits."""
    m, e = np.frexp(x)
    s = np.float32(1 << bits)
    m = np.round(m * s) / s
    return np.ldexp(m, e).astype(np.float32)


_PERM = ([0, 1, 2, 3], [2, 3, 0, 1])   # self-inverse block perms per h

_state = {}


def make_inputs(x):
    xs = np.asarray(x)[:, :, :, 0]                      # (B, C, N) fp32
    nrm = np.sqrt(np.sum(xs * xs, axis=1, keepdims=True))
    xn = xs / np.maximum(nrm, EPS)

    Nsq = float(N) * float(N)
    in_maps = []
    xn64s, thrs = [], []
    for b in range(B):
        xb64 = xn[b].astype(np.float64)
        s = xb64.sum(axis=1)
        M = xb64 @ xb64.T
        sum_g = float(s @ s)
        sum_g2 = float((M * M).sum())
        mean = (2.0 * sum_g - 2.0 * Nsq) / Nsq
        s2 = 4.0 * sum_g2 - 8.0 * sum_g + 4.0 * Nsq
        var = (s2 - Nsq * mean * mean) / (Nsq - 1.0)
        t_b = (mean + PPF_09 * np.sqrt(var) + 2.0) / 2.0
        xn64s.append(xb64)
        thrs.append(t_b)

        nthr_dev = np.full((128, 1), -t_b * OSCALE, np.float32)
        xbr = _round_mant(xn[b].astype(np.float32), MBITS)  # (C, N)
        for h in range(2):
            xloc = xbr.reshape(C, NCHUNK, 4, 128)[:, :, _PERM[h], :]
            xloc = xloc.reshape(C, N)
            xcarr = xloc.reshape(KT, 128, NCHUNK, CW).transpose(2, 1, 0, 3)
            in_maps.append({
                "xc": np.ascontiguousarray(xcarr),
                "nthr": nthr_dev,
            })
    _state["xn64"] = xn64s
    _state["thr"] = thrs
    return in_maps


def assemble(results):
    out = np.empty((2, B * N * N), np.int32)
    iota = np.arange(N, dtype=np.int32)
    neg1 = np.int32(-1)
    for b in range(B):
        dU = np.empty((N, N), np.float32)
        for h in range(2):
            dv = results[2 * b + h]["d"]      # [m, 128, slot, 512] fp8
            for m in range(NCHUNK):
                T = 2 * (m + 1)
                blk = dv[m].astype(np.float32)             # [128, 16, 512]
                if h == 1:
                    blk = blk.reshape(128, 16, 4, 128)[:, :, _PERM[1], :]
                    blk = blk.reshape(128, 16, CW)
                for rb in range(T):
                    a = 4 * (rb // 2) + (rb % 2) + 2 * h
                    slot = rb // 2 + 8 * (rb % 2)
                    dU[a * 128:(a + 1) * 128,
                       m * CW:(m + 1) * CW] = blk[:, slot, :]
        adjU = (dU > 0).astype(np.uint8)
        nearU = np.triu(np.abs(dU) < MARGIN * OSCALE)
        ii, jj = np.nonzero(nearU)
        if ii.size:
            xn64 = _state["xn64"][b]
            g = np.einsum('ci,ci->i', xn64[:, ii], xn64[:, jj])
            adjU[ii, jj] = g > _state["thr"][b]
        adj = np.triu(adjU)
        adj += np.triu(adjU, 1).T
        src = b * N + iota
        out[0, b * N * N:(b + 1) * N * N] = np.where(
            adj, src[:, None], neg1).ravel()
        out[1, b * N * N:(b + 1) * N * N] = np.where(
            adj, src[None, :], neg1).ravel()
    return out


def kernel(x):
    from concourse.bass_utils import run_bass_kernel_spmd

    nc = get_nc()
    in_maps = make_inputs(x)
    res = run_bass_kernel_spmd(nc, in_maps, list(range(NCORES)))
    return assemble(res.results)
